# revision 14
# baseline (speedup 1.0000x reference)
"""Two-layer GAT on 8 TRN2 NeuronCores (Bass/Tile).

Strategy (self-contained; structure derived from edge_index at build time):
- Nodes/dst-segments sharded 8 ways (12500 own-dst nodes per core).
- Per core, its edges are sorted by dst and packed into 128-edge chunks that
  cover <=8 whole dst segments; 16 chunks = one 128-node "region" of a padded
  node space. All per-core structure differences are carried via input
  tensors; the instruction stream is identical (SPMD, one NEFF).
- Per layer: a full feature table ([N,65] = [xs | a_s]) lives in DRAM; each
  core gathers its edges' source rows with indirect DMA (128 rows/inst,
  static stream). The segment softmax is max-free: w = exp(leaky(a_s+a_d)),
  out = (sum w*xs)/(sum w), accumulated per chunk by one PE matmul
  (lhsT = gathered rows [128e,65], rhs = w-valued one-hot [128e,8]) into a
  transposed psum region [65,128]; the ones-column of the rhs-builder gives
  denominators for free (row 64 after the a_s column is overwritten with 1).
- Layer-1 table is computed locally from the replicated x; h is exchanged
  via one AllGather (transposed layout) to build the layer-2 table.
"""
import numpy as np

N = 100000
E = 1600000
D = 128
H = 64
NC = 8
NOWN = N // NC
WIN = 8      # dst nodes per chunk window
CHE = 128    # edges per chunk
RPC = 16     # chunks per region
NEG = 0.2


def _prep(edge_index):
    src_g = edge_index[0].astype(np.int64)
    dst_g = edge_index[1].astype(np.int64)
    cores = []
    for c in range(NC):
        lo, hi = c * NOWN, (c + 1) * NOWN
        sel = np.where((dst_g >= lo) & (dst_g < hi))[0]
        order = np.argsort(dst_g[sel], kind="stable")
        es = src_g[sel][order]
        ed = dst_g[sel][order] - lo
        deg = np.bincount(ed, minlength=NOWN)
        assert deg.max() <= CHE, f"degree {deg.max()} > {CHE}"
        # greedy chunks: <=WIN nodes, <=CHE edges, whole segments only
        chunks = []  # (node_lo, node_hi, edge_lo, edge_hi)
        nlo = 0
        eptr = 0
        cur_nodes = 0
        cur_edges = 0
        nhi = 0
        for n in range(NOWN):
            dg = deg[n]
            if cur_nodes + 1 > WIN or cur_edges + dg > CHE:
                chunks.append((nlo, nhi, eptr, eptr + cur_edges))
                eptr += cur_edges
                nlo = n
                cur_nodes = 0
                cur_edges = 0
            cur_nodes += 1
            cur_edges += dg
            nhi = n + 1
        chunks.append((nlo, nhi, eptr, eptr + cur_edges))
        cores.append(dict(es=es, ed=ed, chunks=chunks))
    R = max((len(c["chunks"]) + RPC - 1) // RPC for c in cores)
    R += (-R) % 4  # multiple of 4 (hT halves, 256-col groups)
    C = R * RPC
    NPAD = R * 128
    gidx1 = np.zeros((NC, 128, C), np.int32)
    dstj = np.full((NC, 128, C), -1.0, np.float32)
    nodemap = np.full((NC, NOWN), -1, np.int64)
    for c in range(NC):
        es, ed, chunks = cores[c]["es"], cores[c]["ed"], cores[c]["chunks"]
        for k, (nlo, nhi, elo, ehi) in enumerate(chunks):
            ne = ehi - elo
            assert ne <= CHE and nhi - nlo <= WIN
            gidx1[c, :ne, k] = es[elo:ehi]
            dstj[c, :ne, k] = (ed[elo:ehi] - nlo).astype(np.float32)
            reg, kk = k // RPC, k % RPC
            rows = 128 * reg + WIN * kk + np.arange(nhi - nlo)
            nodemap[c, nlo:nhi] = rows
    # padded-global id for layer-2 gather; tables use interleaved row order:
    # table1 row of node n = (n%128)*TN1 + n//128; table2 row within owner
    # block = (pr%128)*R + pr//128 for padded row pr.
    TN1 = (((N + 127) // 128 + 7) // 8) * 8
    gidx2 = np.zeros((NC, 128, C), np.int32)
    for c in range(NC):
        g = gidx1[c].astype(np.int64)
        own = g // NOWN
        loc = g % NOWN
        pr = nodemap[own, loc]
        gidx2[c] = (own * NPAD + (pr % 128) * R + pr // 128).astype(np.int32)
    gidx1v = ((gidx1.astype(np.int64) % 128) * TN1 + gidx1.astype(np.int64) // 128).astype(np.int32)
    return cores, R, C, NPAD, gidx1v, gidx2, dstj, nodemap


def _build_and_run(inp, prep):
    import concourse.bass as bass
    import concourse.bacc as bacc
    import concourse.mybir as mybir
    import concourse.tile as tile
    from concourse.bass_utils import run_bass_kernel_spmd

    cores, R, C, NPAD, gidx1, gidx2, dstj, nodemap = prep
    f32, i32 = mybir.dt.float32, mybir.dt.int32
    TN1 = (((N + 127) // 128 + 7) // 8) * 8  # 784 tiles of x_full (mult of 8)
    NF1 = TN1 * 128         # padded table1 rows (interleaved: row = (n%128)*TN1 + n//128)
    HALF = NPAD // 2

    nc = bacc.Bacc("TRN2", target_bir_lowering=False, debug=False, num_devices=NC)
    t_xTf = nc.dram_tensor("xTf", [128, NF1], f32, kind="ExternalInput")
    t_xpT = nc.dram_tensor("xpT", [128, NPAD], f32, kind="ExternalInput")
    t_g1 = nc.dram_tensor("g1", [128, C], i32, kind="ExternalInput")
    t_g2 = nc.dram_tensor("g2", [128, C], i32, kind="ExternalInput")
    t_dj = nc.dram_tensor("dj", [128, C], f32, kind="ExternalInput")
    t_we1 = nc.dram_tensor("we1", [128, 65], f32, kind="ExternalInput")
    t_wl1 = nc.dram_tensor("wl1", [128, 64], f32, kind="ExternalInput")
    t_wd1 = nc.dram_tensor("wd1", [128, 1], f32, kind="ExternalInput")
    t_b1 = nc.dram_tensor("b1", [1, 64], f32, kind="ExternalInput")
    t_we2 = nc.dram_tensor("we2", [64, 65], f32, kind="ExternalInput")
    t_wl2 = nc.dram_tensor("wl2", [64, 64], f32, kind="ExternalInput")
    t_wd2 = nc.dram_tensor("wd2", [64, 1], f32, kind="ExternalInput")
    t_b2 = nc.dram_tensor("b2", [1, 64], f32, kind="ExternalInput")
    t_id = nc.dram_tensor("idm", [128, 128], f32, kind="ExternalInput")
    t_on = nc.dram_tensor("ones1", [1, 128], f32, kind="ExternalInput")
    t_io8 = nc.dram_tensor("iota8", [128, 8], f32, kind="ExternalInput")
    t_out = nc.dram_tensor("outp", [NPAD, 64], f32, kind="ExternalOutput")

    LR = mybir.ActivationFunctionType.Lrelu
    EXP = mybir.ActivationFunctionType.Exp
    RELU = mybir.ActivationFunctionType.Relu
    EQ = mybir.AluOpType.is_equal
    MULT = mybir.AluOpType.mult
    ADD = mybir.AluOpType.add

    with tile.TileContext(nc) as tc:
        with (
            tc.tile_pool(name="dram", bufs=1, space="DRAM") as dpool,
            tc.tile_pool(name="const", bufs=1) as cpool,
        ):
            tb1 = dpool.tile([NF1, 65], f32)
            tb2 = dpool.tile([NC * NPAD, 65], f32)
            QTR = NPAD // 4
            hTdQ = [dpool.tile([64, QTR], f32, name=f"hTdQ{i}", tag=f"hTdQ{i}") for i in range(4)]
            hTgQ = [dpool.tile([NC * 64, QTR], f32, name=f"hTgQ{i}", tag=f"hTgQ{i}") for i in range(4)]

            we1 = cpool.tile([128, 65], f32); nc.sync.dma_start(we1[:], t_we1[:])
            wl1 = cpool.tile([128, 64], f32); nc.sync.dma_start(wl1[:], t_wl1[:])
            wd1 = cpool.tile([128, 1], f32); nc.sync.dma_start(wd1[:], t_wd1[:])
            b1 = cpool.tile([1, 64], f32); nc.sync.dma_start(b1[:], t_b1[:])
            we2 = cpool.tile([64, 65], f32); nc.sync.dma_start(we2[:], t_we2[:])
            wl2 = cpool.tile([64, 64], f32); nc.sync.dma_start(wl2[:], t_wl2[:])
            wd2 = cpool.tile([64, 1], f32); nc.sync.dma_start(wd2[:], t_wd2[:])
            b2 = cpool.tile([1, 64], f32); nc.sync.dma_start(b2[:], t_b2[:])
            idm = cpool.tile([128, 128], f32); nc.sync.dma_start(idm[:], t_id[:])
            on1 = cpool.tile([1, 128], f32); nc.sync.dma_start(on1[:], t_on[:])
            io8 = cpool.tile([128, 8], f32); nc.sync.dma_start(io8[:], t_io8[:])
            g1sb = cpool.tile([128, C], i32); nc.sync.dma_start(g1sb[:], t_g1[:])
            g2sb = cpool.tile([128, C], i32); nc.sync.dma_start(g2sb[:], t_g2[:])
            djsb = cpool.tile([128, C], f32); nc.sync.dma_start(djsb[:], t_dj[:])
            xlad1 = cpool.tile([128, R, 64], f32)
            xlad2 = cpool.tile([128, R, 64], f32)
            ad1 = cpool.tile([128, R], f32)
            ad2 = cpool.tile([128, R], f32)
            hTa = cpool.tile([64, HALF], f32)
            hTb = cpool.tile([64, HALF], f32)

            # ---- phase A: table1 = x_full @ We1 ----
            with (
                tc.tile_pool(name="pa", bufs=3) as pa,
                tc.tile_pool(name="pap", bufs=2, space="PSUM") as pap,
            ):
                for g in range(TN1 // 8):
                    xt = pa.tile([128, 1024], f32, tag="xt")
                    nc.sync.dma_start(xt[:], t_xTf[:, 1024 * g : 1024 * (g + 1)])
                    psa = pap.tile([128, 260], f32, tag="psa")
                    psb = pap.tile([128, 260], f32, tag="psb")
                    for i in range(4):
                        nc.tensor.matmul(out=psa[:, 65 * i : 65 * (i + 1)], lhsT=xt[:, 128 * i : 128 * (i + 1)], rhs=we1[:], start=True, stop=True)
                    for i in range(4):
                        nc.tensor.matmul(out=psb[:, 65 * i : 65 * (i + 1)], lhsT=xt[:, 128 * (4 + i) : 128 * (5 + i)], rhs=we1[:], start=True, stop=True)
                    ot = pa.tile([128, 520], f32, tag="ot")
                    nc.scalar.copy(ot[:, 0:260], psa[:])
                    nc.scalar.copy(ot[:, 260:520], psb[:])
                    # interleaved rows: partition p writes 8 contiguous rows p*TN1 + [8g, 8g+8)
                    nc.sync.dma_start(
                        tb1[:].rearrange("(p t) e -> p t e", p=128)[:, 8 * g : 8 * g + 8, :],
                        ot[:].rearrange("p (t e) -> p t e", e=65),
                    )

            # ---- phase A2: xl/a_d for layer 1 from x_pad ----
            with (
                tc.tile_pool(name="pb", bufs=3) as pb,
                tc.tile_pool(name="pbp", bufs=2, space="PSUM") as pbp,
            ):
                for r in range(R):
                    xt = pb.tile([128, 128], f32, tag="xt2")
                    nc.sync.dma_start(xt[:], t_xpT[:, 128 * r : 128 * (r + 1)])
                    ps = pbp.tile([128, 64], f32, tag="psl")
                    nc.tensor.matmul(out=ps[:], lhsT=xt[:], rhs=wl1[:], start=True, stop=True)
                    nc.tensor.matmul(out=ps[:], lhsT=on1[:], rhs=b1[:], start=False, stop=True)
                    nc.scalar.copy(xlad1[:, r, :], ps[:])
                    psd = pbp.tile([128, 1], f32, tag="psd")
                    nc.tensor.matmul(out=psd[:], lhsT=xt[:], rhs=wd1[:], start=True, stop=True)
                    nc.scalar.copy(ad1[:, r : r + 1], psd[:])

            # ---- gather/softmax/scatter layer ----
            def gat_layer(table, gsb, adt, xlad, out_h):
                # out_h(r, h_sb[128,64]) -> emitted per region
                with (
                    tc.tile_pool(name="pg", bufs=6) as pg,
                    tc.tile_pool(name="pgp", bufs=3, space="PSUM") as pgp,
                    tc.tile_pool(name="pgp1", bufs=2, space="PSUM") as pgp1,
                ):
                    for r in range(R):
                        G = pg.tile([128, RPC, 65], f32, tag="G")
                        for k in range(RPC):
                            nc.gpsimd.indirect_dma_start(
                                out=G[:, k, :], out_offset=None, in_=table[:],
                                in_offset=bass.IndirectOffsetOnAxis(ap=gsb[:, RPC * r + k : RPC * r + k + 1], axis=0),
                            )
                        # a_d broadcast: transpose column -> row, ones-bcast
                        pst = pgp1.tile([128, 128], f32, tag="adB")
                        nc.tensor.transpose(out=pst[0:1, :], in_=adt[:, r : r + 1], identity=idm[:])
                        adrow = pg.tile([1, 128], f32, tag="adrow")
                        nc.scalar.copy(adrow[:], pst[0:1, :])
                        adB = pgp1.tile([128, 128], f32, tag="adB")
                        nc.tensor.matmul(out=adB[:], lhsT=on1[:], rhs=adrow[:], start=True, stop=True)
                        # grid: w-one-hot
                        grid = pg.tile([128, RPC, 8], f32, tag="grid")
                        nc.vector.tensor_tensor(
                            out=grid[:], in0=G[:, :, 64:65].to_broadcast([128, RPC, 8]),
                            in1=adB[:].rearrange("p (a b) -> p a b", b=8), op=ADD,
                        )
                        grid2 = pg.tile([128, RPC, 8], f32, tag="grid2")
                        nc.vector.tensor_scalar(grid2[:], grid[:], NEG, None, op0=MULT)
                        nc.vector.tensor_tensor(out=grid[:], in0=grid[:], in1=grid2[:], op=mybir.AluOpType.max)
                        nc.scalar.activation(grid[:], grid[:], EXP)
                        ind = pg.tile([128, RPC, 8], f32, tag="ind")
                        nc.vector.tensor_tensor(
                            out=ind[:], in0=djsb[:, RPC * r : RPC * (r + 1)].unsqueeze(2).to_broadcast([128, RPC, 8]),
                            in1=io8[:].unsqueeze(1).to_broadcast([128, RPC, 8]), op=EQ,
                        )
                        nc.vector.tensor_tensor(out=ind[:], in0=ind[:], in1=grid[:], op=MULT)
                        nc.vector.memset(G[:, :, 64:65], 1.0)
                        acc = pgp.tile([65, 128], f32, tag="acc")
                        for k in range(RPC):
                            nc.tensor.matmul(
                                out=acc[:, 8 * k : 8 * (k + 1)], lhsT=G[:, k, :], rhs=ind[:, k, :],
                                start=True, stop=True,
                            )
                        accs = pg.tile([65, 128], f32, tag="accs")
                        nc.scalar.copy(accs[:], acc[:])
                        accT = pgp.tile([128, 65], f32, tag="accT")
                        nc.tensor.transpose(out=accT[:], in_=accs[:], identity=idm[0:65, 0:65])
                        den = pg.tile([128, 1], f32, tag="den")
                        nc.vector.tensor_scalar(den[:], accT[:, 64:65], 1e-16, None, op0=ADD)
                        rcp = pg.tile([128, 1], f32, tag="rcp")
                        nc.vector.reciprocal(rcp[:], den[:])
                        hsb = pg.tile([128, 64], f32, tag="hsb")
                        nc.vector.tensor_scalar(hsb[:], accT[:, 0:64], rcp[:], None, op0=MULT)
                        nc.vector.tensor_tensor(out=hsb[:], in0=hsb[:], in1=xlad[:, r, :], op=ADD)
                        out_h(r, hsb, pg, pgp1)

            # layer 1: h = relu(...); also build hT in SBUF
            def finish1(r, hsb, pg, pgp1):
                nc.scalar.activation(hsb[:], hsb[:], RELU)
                psT = pgp1.tile([128, 128], f32, tag="adB")
                rh, rr = (0, r) if r < R // 2 else (1, r - R // 2)
                dst = hTa if rh == 0 else hTb
                nc.tensor.transpose(out=psT[0:64, :], in_=hsb[:], identity=idm[:])
                nc.scalar.copy(dst[:, 128 * rr : 128 * (rr + 1)], psT[0:64, :])
                for q in range(3):
                    if r == (q + 1) * (R // 4) - 1:
                        srcq = (hTa if q < 2 else hTb)[:, (q % 2) * QTR : (q % 2) * QTR + QTR]
                        nc.sync.dma_start(hTdQ[q][:], srcq)
                        nc.gpsimd.collective_compute(
                            "AllGather", mybir.AluOpType.bypass,
                            replica_groups=[list(range(NC))],
                            ins=[hTdQ[q].opt()], outs=[hTgQ[q].opt()],
                        )

            gat_layer(tb1, g1sb, ad1, xlad1, finish1)

            # ---- phase C: allgather last quarter of hT ----
            nc.sync.dma_start(hTdQ[3][:], hTb[:, QTR : 2 * QTR])
            nc.gpsimd.collective_compute(
                "AllGather", mybir.AluOpType.bypass,
                replica_groups=[list(range(NC))],
                ins=[hTdQ[3].opt()], outs=[hTgQ[3].opt()],
            )

            # ---- phase D: table2 + xl/a_d layer 2 ----
            with (
                tc.tile_pool(name="pd", bufs=3) as pd,
                tc.tile_pool(name="pdp", bufs=2, space="PSUM") as pdp,
            ):
                NG3 = QTR // 384  # groups of 3 tiles per quarter
                assert NG3 * 384 == QTR, (QTR, NG3)
                for q in range(4):
                    for o in range(NC):
                        for g in range(NG3):
                            ht = pd.tile([64, 384], f32, tag="ht")
                            nc.sync.dma_start(ht[:], hTgQ[q][64 * o : 64 * o + 64, 384 * g : 384 * (g + 1)])
                            ps = pdp.tile([128, 195], f32, tag="ps2")
                            for i in range(3):
                                nc.tensor.matmul(out=ps[:, 65 * i : 65 * (i + 1)], lhsT=ht[:, 128 * i : 128 * (i + 1)], rhs=we2[:], start=True, stop=True)
                            ot = pd.tile([128, 195], f32, tag="ot2")
                            nc.scalar.copy(ot[:], ps[:])
                            tg = q * (R // 4) + 3 * g
                            nc.sync.dma_start(
                                tb2[:].rearrange("(o p t) e -> o p t e", p=128, t=R)[o, :, tg : tg + 3, :],
                                ot[:].rearrange("p (a e) -> p a e", e=65),
                            )
                for r in range(R):
                    rh, rr = (0, r) if r < R // 2 else (1, r - R // 2)
                    lh = (hTa if rh == 0 else hTb)[:, 128 * rr : 128 * (rr + 1)]
                    ps = pdp.tile([128, 64], f32, tag="psl2")
                    nc.tensor.matmul(out=ps[:], lhsT=lh, rhs=wl2[:], start=True, stop=True)
                    nc.tensor.matmul(out=ps[:], lhsT=on1[:], rhs=b2[:], start=False, stop=True)
                    nc.scalar.copy(xlad2[:, r, :], ps[:])
                    psd = pdp.tile([128, 1], f32, tag="psd2")
                    nc.tensor.matmul(out=psd[:], lhsT=lh, rhs=wd2[:], start=True, stop=True)
                    nc.scalar.copy(ad2[:, r : r + 1], psd[:])

            # layer 2: out rows
            def finish2(r, hsb, pg, pgp1):
                nc.sync.dma_start(t_out[128 * r : 128 * (r + 1), :], hsb[:])

            gat_layer(tb2, g2sb, ad2, xlad2, finish2)

    nc.finalize()

    in_maps = []
    for c in range(NC):
        in_maps.append({
            "xTf": inp["xTf"], "xpT": inp["xpT"][c], "g1": gidx1[c], "g2": gidx2[c],
            "dj": dstj[c], "we1": inp["we1"], "wl1": inp["wl1"], "wd1": inp["wd1"],
            "b1": inp["b1"], "we2": inp["we2"], "wl2": inp["wl2"], "wd2": inp["wd2"],
            "b2": inp["b2"], "idm": np.eye(128, dtype=np.float32),
            "ones1": np.ones((1, 128), np.float32),
            "iota8": np.tile(np.arange(8, dtype=np.float32), (128, 1)),
        })
    globals()["_LAST_NC"] = nc
    globals()["_LAST_INMAPS"] = in_maps
    res = run_bass_kernel_spmd(nc, in_maps, core_ids=list(range(NC)))
    return [r["outp"] for r in res.results]


def kernel(**inputs):
    x = np.asarray(inputs["x"], np.float32)
    ei = np.asarray(inputs["edge_index"])
    prep = _prep(ei)
    cores, R, C, NPAD, gidx1, gidx2, dstj, nodemap = prep

    W_src1 = np.asarray(inputs["W_src1"], np.float32)
    W_dst1 = np.asarray(inputs["W_dst1"], np.float32)
    att_src1 = np.asarray(inputs["att_src1"], np.float32)
    att_dst1 = np.asarray(inputs["att_dst1"], np.float32)
    bias1 = np.asarray(inputs["bias1"], np.float32)
    Wl1 = np.asarray(inputs["Wl1"], np.float32)
    bl1 = np.asarray(inputs["bl1"], np.float32)
    W_src2 = np.asarray(inputs["W_src2"], np.float32)
    W_dst2 = np.asarray(inputs["W_dst2"], np.float32)
    att_src2 = np.asarray(inputs["att_src2"], np.float32)
    att_dst2 = np.asarray(inputs["att_dst2"], np.float32)
    bias2 = np.asarray(inputs["bias2"], np.float32)
    Wl2 = np.asarray(inputs["Wl2"], np.float32)
    bl2 = np.asarray(inputs["bl2"], np.float32)

    TN1 = (((N + 127) // 128 + 7) // 8) * 8
    NF1 = TN1 * 128
    xf = np.zeros((NF1, D), np.float32)
    xf[:N] = x
    xTf = np.ascontiguousarray(xf.T)  # [128, NF1]
    xpT = np.zeros((NC, D, NPAD), np.float32)
    for c in range(NC):
        xp = np.zeros((NPAD, D), np.float32)
        rows = nodemap[c]
        xp[rows] = x[c * NOWN : (c + 1) * NOWN]
        xpT[c] = xp.T
    inp = dict(
        xTf=xTf, xpT=xpT,
        we1=np.concatenate([W_src1, (W_src1 @ att_src1)[:, None]], 1).astype(np.float32),
        wl1=Wl1, wd1=(W_dst1 @ att_dst1)[:, None].astype(np.float32),
        b1=(bias1 + bl1)[None, :].astype(np.float32),
        we2=np.concatenate([W_src2, (W_src2 @ att_src2)[:, None]], 1).astype(np.float32),
        wl2=Wl2, wd2=(W_dst2 @ att_dst2)[:, None].astype(np.float32),
        b2=(bias2 + bl2)[None, :].astype(np.float32),
    )
    outs = _build_and_run(inp, prep)
    full = np.zeros((N, H), np.float32)
    for c in range(NC):
        full[c * NOWN : (c + 1) * NOWN] = outs[c][nodemap[c]]
    return full


# revision 17
# speedup vs baseline: 10.5099x; 10.5099x over previous
"""Two-layer GAT on 8 TRN2 NeuronCores (Bass/Tile).

Strategy (self-contained; structure derived from edge_index at build time):
- Nodes/dst-segments sharded 8 ways (12500 own-dst nodes per core).
- Per core, its edges are sorted by dst and packed into 128-edge chunks that
  cover <=8 whole dst segments; 16 chunks = one 128-node "region" of a padded
  node space. All per-core structure differences are carried via input
  tensors; the instruction stream is identical (SPMD, one NEFF).
- Per layer: a full feature table ([N,65] = [xs | a_s]) lives in DRAM; each
  core gathers its edges' source rows with indirect DMA (128 rows/inst,
  static stream). The segment softmax is max-free: w = exp(leaky(a_s+a_d)),
  out = (sum w*xs)/(sum w), accumulated per chunk by one PE matmul
  (lhsT = gathered rows [128e,65], rhs = w-valued one-hot [128e,8]) into a
  transposed psum region [65,128]; the ones-column of the rhs-builder gives
  denominators for free (row 64 after the a_s column is overwritten with 1).
- Layer-1 table is computed locally from the replicated x; h is exchanged
  via one AllGather (transposed layout) to build the layer-2 table.
"""
import numpy as np

N = 100000
E = 1600000
D = 128
H = 64
NC = 8
NOWN = N // NC
WIN = 8      # dst nodes per chunk window
CHE = 128    # edges per chunk
RPC = 16     # chunks per region
NEG = 0.2


def _prep(edge_index):
    src_g = edge_index[0].astype(np.int64)
    dst_g = edge_index[1].astype(np.int64)
    cores = []
    for c in range(NC):
        lo, hi = c * NOWN, (c + 1) * NOWN
        sel = np.where((dst_g >= lo) & (dst_g < hi))[0]
        order = np.argsort(dst_g[sel], kind="stable")
        es = src_g[sel][order]
        ed = dst_g[sel][order] - lo
        deg = np.bincount(ed, minlength=NOWN)
        assert deg.max() <= CHE, f"degree {deg.max()} > {CHE}"
        # greedy chunks: <=WIN nodes, <=CHE edges, whole segments only
        chunks = []  # (node_lo, node_hi, edge_lo, edge_hi)
        nlo = 0
        eptr = 0
        cur_nodes = 0
        cur_edges = 0
        nhi = 0
        for n in range(NOWN):
            dg = deg[n]
            if cur_nodes + 1 > WIN or cur_edges + dg > CHE:
                chunks.append((nlo, nhi, eptr, eptr + cur_edges))
                eptr += cur_edges
                nlo = n
                cur_nodes = 0
                cur_edges = 0
            cur_nodes += 1
            cur_edges += dg
            nhi = n + 1
        chunks.append((nlo, nhi, eptr, eptr + cur_edges))
        cores.append(dict(es=es, ed=ed, chunks=chunks))
    R = max((len(c["chunks"]) + RPC - 1) // RPC for c in cores)
    R += (-R) % 4  # multiple of 4 (hT halves, 256-col groups)
    C = R * RPC
    NPAD = R * 128
    gidx1 = np.zeros((NC, 128, C), np.int32)
    dstj = np.full((NC, 128, C), -1.0, np.float32)
    nodemap = np.full((NC, NOWN), -1, np.int64)
    for c in range(NC):
        es, ed, chunks = cores[c]["es"], cores[c]["ed"], cores[c]["chunks"]
        for k, (nlo, nhi, elo, ehi) in enumerate(chunks):
            ne = ehi - elo
            assert ne <= CHE and nhi - nlo <= WIN
            gidx1[c, :ne, k] = es[elo:ehi]
            dstj[c, :ne, k] = (ed[elo:ehi] - nlo).astype(np.float32)
            reg, kk = k // RPC, k % RPC
            rows = 128 * reg + WIN * kk + np.arange(nhi - nlo)
            nodemap[c, nlo:nhi] = rows
    # padded-global id for layer-2 gather; tables use interleaved row order:
    # table1 row of node n = (n%128)*TN1 + n//128; table2 row within owner
    # block = (pr%128)*R + pr//128 for padded row pr.
    TN1 = (((N + 127) // 128 + 7) // 8) * 8
    gidx2 = np.zeros((NC, 128, C), np.int32)
    for c in range(NC):
        g = gidx1[c].astype(np.int64)
        own = g // NOWN
        loc = g % NOWN
        pr = nodemap[own, loc]
        gidx2[c] = (own * NPAD + (pr % 128) * R + pr // 128).astype(np.int32)
    gidx1v = ((gidx1.astype(np.int64) % 128) * TN1 + gidx1.astype(np.int64) // 128).astype(np.int32)
    return cores, R, C, NPAD, gidx1v, gidx2, dstj, nodemap


def _build_and_run(inp, prep):
    import concourse.bass as bass
    import concourse.bacc as bacc
    import concourse.mybir as mybir
    import concourse.tile as tile
    from concourse.bass_utils import run_bass_kernel_spmd

    cores, R, C, NPAD, gidx1, gidx2, dstj, nodemap = prep
    f32, i32 = mybir.dt.float32, mybir.dt.int32
    TN1 = (((N + 127) // 128 + 7) // 8) * 8  # 784 tiles of x_full (mult of 8)
    NF1 = TN1 * 128         # padded table1 rows (interleaved: row = (n%128)*TN1 + n//128)
    HALF = NPAD // 2

    nc = bacc.Bacc("TRN2", target_bir_lowering=False, debug=False, num_devices=NC)
    t_xTf = nc.dram_tensor("xTf", [128, NF1], f32, kind="ExternalInput")
    t_xpT = nc.dram_tensor("xpT", [128, NPAD], f32, kind="ExternalInput")
    t_g1 = nc.dram_tensor("g1", [128, C], i32, kind="ExternalInput")
    t_g2 = nc.dram_tensor("g2", [128, C], i32, kind="ExternalInput")
    t_dj = nc.dram_tensor("dj", [128, C], f32, kind="ExternalInput")
    t_we1 = nc.dram_tensor("we1", [128, 65], f32, kind="ExternalInput")
    t_wl1 = nc.dram_tensor("wl1", [128, 64], f32, kind="ExternalInput")
    t_wd1 = nc.dram_tensor("wd1", [128, 1], f32, kind="ExternalInput")
    t_b1 = nc.dram_tensor("b1", [1, 64], f32, kind="ExternalInput")
    t_we2 = nc.dram_tensor("we2", [64, 65], f32, kind="ExternalInput")
    t_wl2 = nc.dram_tensor("wl2", [64, 64], f32, kind="ExternalInput")
    t_wd2 = nc.dram_tensor("wd2", [64, 1], f32, kind="ExternalInput")
    t_b2 = nc.dram_tensor("b2", [1, 64], f32, kind="ExternalInput")
    t_id = nc.dram_tensor("idm", [128, 128], f32, kind="ExternalInput")
    t_on = nc.dram_tensor("ones1", [1, 128], f32, kind="ExternalInput")
    t_io8 = nc.dram_tensor("iota8", [128, 8], f32, kind="ExternalInput")
    t_out = nc.dram_tensor("outp", [NPAD, 64], f32, kind="ExternalOutput")

    LR = mybir.ActivationFunctionType.Lrelu
    EXP = mybir.ActivationFunctionType.Exp
    RELU = mybir.ActivationFunctionType.Relu
    EQ = mybir.AluOpType.is_equal
    MULT = mybir.AluOpType.mult
    ADD = mybir.AluOpType.add

    with tile.TileContext(nc) as tc:
        with (
            tc.tile_pool(name="dram", bufs=1, space="DRAM") as dpool,
            tc.tile_pool(name="const", bufs=1) as cpool,
        ):
            tb1 = dpool.tile([NF1, 65], f32)
            tb2 = dpool.tile([NC * NPAD, 65], f32)
            QTR = NPAD // 4
            hTdQ = [dpool.tile([64, QTR], f32, name=f"hTdQ{i}", tag=f"hTdQ{i}") for i in range(4)]
            hTgQ = [dpool.tile([NC * 64, QTR], f32, name=f"hTgQ{i}", tag=f"hTgQ{i}") for i in range(4)]

            we1 = cpool.tile([128, 65], f32); nc.sync.dma_start(we1[:], t_we1[:])
            wl1 = cpool.tile([128, 64], f32); nc.sync.dma_start(wl1[:], t_wl1[:])
            wd1 = cpool.tile([128, 1], f32); nc.sync.dma_start(wd1[:], t_wd1[:])
            b1 = cpool.tile([1, 64], f32); nc.sync.dma_start(b1[:], t_b1[:])
            we2 = cpool.tile([64, 65], f32); nc.sync.dma_start(we2[:], t_we2[:])
            wl2 = cpool.tile([64, 64], f32); nc.sync.dma_start(wl2[:], t_wl2[:])
            wd2 = cpool.tile([64, 1], f32); nc.sync.dma_start(wd2[:], t_wd2[:])
            b2 = cpool.tile([1, 64], f32); nc.sync.dma_start(b2[:], t_b2[:])
            idm = cpool.tile([128, 128], f32); nc.sync.dma_start(idm[:], t_id[:])
            on1 = cpool.tile([1, 128], f32); nc.sync.dma_start(on1[:], t_on[:])
            io8 = cpool.tile([128, 8], f32); nc.sync.dma_start(io8[:], t_io8[:])
            g1sb = cpool.tile([128, C], i32); nc.sync.dma_start(g1sb[:], t_g1[:])
            g2sb = cpool.tile([128, C], i32); nc.sync.dma_start(g2sb[:], t_g2[:])
            djsb = cpool.tile([128, C], f32); nc.sync.dma_start(djsb[:], t_dj[:])
            xlad1 = cpool.tile([128, R, 64], f32)
            xlad2 = cpool.tile([128, R, 64], f32)
            ad1 = cpool.tile([128, R], f32)
            ad2 = cpool.tile([128, R], f32)
            hTa = cpool.tile([64, HALF], f32)
            hTb = cpool.tile([64, HALF], f32)

            # ---- phase A: table1 = x_full @ We1 ----
            with (
                tc.tile_pool(name="pa", bufs=3) as pa,
                tc.tile_pool(name="pap", bufs=2, space="PSUM") as pap,
            ):
                for g in range(TN1 // 8):
                    xt = pa.tile([128, 1024], f32, tag="xt")
                    nc.sync.dma_start(xt[:], t_xTf[:, 1024 * g : 1024 * (g + 1)])
                    psa = pap.tile([128, 260], f32, tag="psa")
                    psb = pap.tile([128, 260], f32, tag="psb")
                    for i in range(4):
                        nc.tensor.matmul(out=psa[:, 65 * i : 65 * (i + 1)], lhsT=xt[:, 128 * i : 128 * (i + 1)], rhs=we1[:], start=True, stop=True)
                    for i in range(4):
                        nc.tensor.matmul(out=psb[:, 65 * i : 65 * (i + 1)], lhsT=xt[:, 128 * (4 + i) : 128 * (5 + i)], rhs=we1[:], start=True, stop=True)
                    ot = pa.tile([128, 520], f32, tag="ot")
                    nc.scalar.copy(ot[:, 0:260], psa[:])
                    nc.scalar.copy(ot[:, 260:520], psb[:])
                    # interleaved rows: partition p writes 8 contiguous rows p*TN1 + [8g, 8g+8)
                    nc.sync.dma_start(
                        tb1[:].rearrange("(p t) e -> p t e", p=128)[:, 8 * g : 8 * g + 8, :],
                        ot[:].rearrange("p (t e) -> p t e", e=65),
                    )

            # ---- phase A2: xl/a_d for layer 1 from x_pad ----
            with (
                tc.tile_pool(name="pb", bufs=3) as pb,
                tc.tile_pool(name="pbp", bufs=2, space="PSUM") as pbp,
            ):
                for r in range(R):
                    xt = pb.tile([128, 128], f32, tag="xt2")
                    nc.sync.dma_start(xt[:], t_xpT[:, 128 * r : 128 * (r + 1)])
                    ps = pbp.tile([128, 64], f32, tag="psl")
                    nc.tensor.matmul(out=ps[:], lhsT=xt[:], rhs=wl1[:], start=True, stop=True)
                    nc.tensor.matmul(out=ps[:], lhsT=on1[:], rhs=b1[:], start=False, stop=True)
                    nc.scalar.copy(xlad1[:, r, :], ps[:])
                    psd = pbp.tile([128, 1], f32, tag="psd")
                    nc.tensor.matmul(out=psd[:], lhsT=xt[:], rhs=wd1[:], start=True, stop=True)
                    nc.scalar.copy(ad1[:, r : r + 1], psd[:])

            # ---- gather/softmax/scatter layer ----
            def gat_layer(table, gsb, adt, xlad, out_h):
                # out_h(r, h_sb[128,64]) -> emitted per region
                with (
                    tc.tile_pool(name="pg", bufs=6) as pg,
                    tc.tile_pool(name="pgp", bufs=3, space="PSUM") as pgp,
                    tc.tile_pool(name="pgp1", bufs=2, space="PSUM") as pgp1,
                ):
                    SBR = 4  # regions per batch (grid ops batched)
                    for rb in range(R // SBR):
                        G = pg.tile([128, SBR * RPC, 65], f32, tag="G", bufs=2)
                        for k in range(SBR * RPC):
                            nc.gpsimd.indirect_dma_start(
                                out=G[:, k, :], out_offset=None, in_=table[:],
                                in_offset=bass.IndirectOffsetOnAxis(ap=gsb[:, SBR * RPC * rb + k : SBR * RPC * rb + k + 1], axis=0),
                            )
                        # a_d broadcast for the 4 regions -> one [128, 512] psum
                        adB = pgp1.tile([128, SBR * 128], f32, tag="adB")
                        for q in range(SBR):
                            r = SBR * rb + q
                            pst = pgp.tile([1, 128], f32, tag="pstT", bufs=1)
                            nc.tensor.transpose(out=pst[0:1, :], in_=adt[:, r : r + 1], identity=idm[:])
                            adrow = pg.tile([1, 128], f32, tag="adrow")
                            nc.scalar.copy(adrow[:], pst[0:1, :])
                            nc.tensor.matmul(out=adB[:, 128 * q : 128 * (q + 1)], lhsT=on1[:], rhs=adrow[:], start=True, stop=True)
                        # grid: w-one-hot, batched over 4 regions
                        grid = pg.tile([128, SBR * RPC, 8], f32, tag="grid", bufs=2)
                        nc.vector.tensor_tensor(
                            out=grid[:], in0=G[:, :, 64:65].to_broadcast([128, SBR * RPC, 8]),
                            in1=adB[:].rearrange("p (a b) -> p a b", b=8), op=ADD,
                        )
                        grid2 = pg.tile([128, SBR * RPC, 8], f32, tag="grid2", bufs=2)
                        nc.vector.tensor_scalar(grid2[:], grid[:], NEG, None, op0=MULT)
                        nc.vector.tensor_tensor(out=grid[:], in0=grid[:], in1=grid2[:], op=mybir.AluOpType.max)
                        nc.scalar.activation(grid[:], grid[:], EXP)
                        ind = pg.tile([128, SBR * RPC, 8], f32, tag="ind", bufs=2)
                        nc.vector.tensor_tensor(
                            out=ind[:], in0=djsb[:, SBR * RPC * rb : SBR * RPC * (rb + 1)].unsqueeze(2).to_broadcast([128, SBR * RPC, 8]),
                            in1=io8[:].unsqueeze(1).to_broadcast([128, SBR * RPC, 8]), op=EQ,
                        )
                        nc.vector.tensor_tensor(out=ind[:], in0=ind[:], in1=grid[:], op=MULT)
                        nc.vector.memset(G[:, :, 64:65], 1.0)
                        for q in range(SBR):
                            r = SBR * rb + q
                            acc = pgp.tile([65, 128], f32, tag="acc")
                            for k in range(RPC):
                                nc.tensor.matmul(
                                    out=acc[:, 8 * k : 8 * (k + 1)], lhsT=G[:, RPC * q + k, :], rhs=ind[:, RPC * q + k, :],
                                    start=True, stop=True,
                                )
                            accs = pg.tile([65, 128], f32, tag="accs")
                            nc.scalar.copy(accs[:], acc[:])
                            accT = pgp.tile([128, 65], f32, tag="accT", bufs=2)
                            nc.tensor.transpose(out=accT[:], in_=accs[:], identity=idm[0:65, 0:65])
                            den = pg.tile([128, 1], f32, tag="den")
                            nc.vector.tensor_scalar(den[:], accT[:, 64:65], 1e-16, None, op0=ADD)
                            rcp = pg.tile([128, 1], f32, tag="rcp")
                            nc.vector.reciprocal(rcp[:], den[:])
                            hsb = pg.tile([128, 64], f32, tag="hsb")
                            nc.vector.tensor_scalar(hsb[:], accT[:, 0:64], rcp[:], None, op0=MULT)
                            nc.vector.tensor_tensor(out=hsb[:], in0=hsb[:], in1=xlad[:, r, :], op=ADD)
                            out_h(r, hsb, pg, pgp1)

            # layer 1: h = relu(...); also build hT in SBUF
            def finish1(r, hsb, pg, pgp1):
                nc.scalar.activation(hsb[:], hsb[:], RELU)
                psT = pgp1.tile([128, 128], f32, tag="adB")
                rh, rr = (0, r) if r < R // 2 else (1, r - R // 2)
                dst = hTa if rh == 0 else hTb
                nc.tensor.transpose(out=psT[0:64, :], in_=hsb[:], identity=idm[:])
                nc.scalar.copy(dst[:, 128 * rr : 128 * (rr + 1)], psT[0:64, :])
                for q in range(3):
                    if r == (q + 1) * (R // 4) - 1:
                        srcq = (hTa if q < 2 else hTb)[:, (q % 2) * QTR : (q % 2) * QTR + QTR]
                        nc.sync.dma_start(hTdQ[q][:], srcq)
                        nc.gpsimd.collective_compute(
                            "AllGather", mybir.AluOpType.bypass,
                            replica_groups=[list(range(NC))],
                            ins=[hTdQ[q].opt()], outs=[hTgQ[q].opt()],
                        )

            gat_layer(tb1, g1sb, ad1, xlad1, finish1)

            # ---- phase C: allgather last quarter of hT ----
            nc.sync.dma_start(hTdQ[3][:], hTb[:, QTR : 2 * QTR])
            nc.gpsimd.collective_compute(
                "AllGather", mybir.AluOpType.bypass,
                replica_groups=[list(range(NC))],
                ins=[hTdQ[3].opt()], outs=[hTgQ[3].opt()],
            )

            # ---- phase D: table2 + xl/a_d layer 2 ----
            with (
                tc.tile_pool(name="pd", bufs=3) as pd,
                tc.tile_pool(name="pdp", bufs=2, space="PSUM") as pdp,
            ):
                NG3 = QTR // 384  # groups of 3 tiles per quarter
                assert NG3 * 384 == QTR, (QTR, NG3)
                for q in range(4):
                    for o in range(NC):
                        for g in range(NG3):
                            ht = pd.tile([64, 384], f32, tag="ht")
                            nc.sync.dma_start(ht[:], hTgQ[q][64 * o : 64 * o + 64, 384 * g : 384 * (g + 1)])
                            ps = pdp.tile([128, 195], f32, tag="ps2")
                            for i in range(3):
                                nc.tensor.matmul(out=ps[:, 65 * i : 65 * (i + 1)], lhsT=ht[:, 128 * i : 128 * (i + 1)], rhs=we2[:], start=True, stop=True)
                            ot = pd.tile([128, 195], f32, tag="ot2")
                            nc.scalar.copy(ot[:], ps[:])
                            tg = q * (R // 4) + 3 * g
                            nc.sync.dma_start(
                                tb2[:].rearrange("(o p t) e -> o p t e", p=128, t=R)[o, :, tg : tg + 3, :],
                                ot[:].rearrange("p (a e) -> p a e", e=65),
                            )
                for r in range(R):
                    rh, rr = (0, r) if r < R // 2 else (1, r - R // 2)
                    lh = (hTa if rh == 0 else hTb)[:, 128 * rr : 128 * (rr + 1)]
                    ps = pdp.tile([128, 64], f32, tag="psl2")
                    nc.tensor.matmul(out=ps[:], lhsT=lh, rhs=wl2[:], start=True, stop=True)
                    nc.tensor.matmul(out=ps[:], lhsT=on1[:], rhs=b2[:], start=False, stop=True)
                    nc.scalar.copy(xlad2[:, r, :], ps[:])
                    psd = pdp.tile([128, 1], f32, tag="psd2")
                    nc.tensor.matmul(out=psd[:], lhsT=lh, rhs=wd2[:], start=True, stop=True)
                    nc.scalar.copy(ad2[:, r : r + 1], psd[:])

            # layer 2: out rows
            def finish2(r, hsb, pg, pgp1):
                nc.sync.dma_start(t_out[128 * r : 128 * (r + 1), :], hsb[:])

            gat_layer(tb2, g2sb, ad2, xlad2, finish2)

    nc.finalize()

    in_maps = []
    for c in range(NC):
        in_maps.append({
            "xTf": inp["xTf"], "xpT": inp["xpT"][c], "g1": gidx1[c], "g2": gidx2[c],
            "dj": dstj[c], "we1": inp["we1"], "wl1": inp["wl1"], "wd1": inp["wd1"],
            "b1": inp["b1"], "we2": inp["we2"], "wl2": inp["wl2"], "wd2": inp["wd2"],
            "b2": inp["b2"], "idm": np.eye(128, dtype=np.float32),
            "ones1": np.ones((1, 128), np.float32),
            "iota8": np.tile(np.arange(8, dtype=np.float32), (128, 1)),
        })
    globals()["_LAST_NC"] = nc
    globals()["_LAST_INMAPS"] = in_maps
    res = run_bass_kernel_spmd(nc, in_maps, core_ids=list(range(NC)))
    return [r["outp"] for r in res.results]


def kernel(**inputs):
    x = np.asarray(inputs["x"], np.float32)
    ei = np.asarray(inputs["edge_index"])
    prep = _prep(ei)
    cores, R, C, NPAD, gidx1, gidx2, dstj, nodemap = prep

    W_src1 = np.asarray(inputs["W_src1"], np.float32)
    W_dst1 = np.asarray(inputs["W_dst1"], np.float32)
    att_src1 = np.asarray(inputs["att_src1"], np.float32)
    att_dst1 = np.asarray(inputs["att_dst1"], np.float32)
    bias1 = np.asarray(inputs["bias1"], np.float32)
    Wl1 = np.asarray(inputs["Wl1"], np.float32)
    bl1 = np.asarray(inputs["bl1"], np.float32)
    W_src2 = np.asarray(inputs["W_src2"], np.float32)
    W_dst2 = np.asarray(inputs["W_dst2"], np.float32)
    att_src2 = np.asarray(inputs["att_src2"], np.float32)
    att_dst2 = np.asarray(inputs["att_dst2"], np.float32)
    bias2 = np.asarray(inputs["bias2"], np.float32)
    Wl2 = np.asarray(inputs["Wl2"], np.float32)
    bl2 = np.asarray(inputs["bl2"], np.float32)

    TN1 = (((N + 127) // 128 + 7) // 8) * 8
    NF1 = TN1 * 128
    xf = np.zeros((NF1, D), np.float32)
    xf[:N] = x
    xTf = np.ascontiguousarray(xf.T)  # [128, NF1]
    xpT = np.zeros((NC, D, NPAD), np.float32)
    for c in range(NC):
        xp = np.zeros((NPAD, D), np.float32)
        rows = nodemap[c]
        xp[rows] = x[c * NOWN : (c + 1) * NOWN]
        xpT[c] = xp.T
    inp = dict(
        xTf=xTf, xpT=xpT,
        we1=np.concatenate([W_src1, (W_src1 @ att_src1)[:, None]], 1).astype(np.float32),
        wl1=Wl1, wd1=(W_dst1 @ att_dst1)[:, None].astype(np.float32),
        b1=(bias1 + bl1)[None, :].astype(np.float32),
        we2=np.concatenate([W_src2, (W_src2 @ att_src2)[:, None]], 1).astype(np.float32),
        wl2=Wl2, wd2=(W_dst2 @ att_dst2)[:, None].astype(np.float32),
        b2=(bias2 + bl2)[None, :].astype(np.float32),
    )
    outs = _build_and_run(inp, prep)
    full = np.zeros((N, H), np.float32)
    for c in range(NC):
        full[c * NOWN : (c + 1) * NOWN] = outs[c][nodemap[c]]
    return full
